# revision 9
# baseline (speedup 1.0000x reference)
"""Trainium2 Bass kernel for a dense transformer block.

Problem: B=8, T=2048, DIM=384, 6 heads (hd=64), FFN hidden 768, causal
attention, RMSNorm (eps 1e-6), exact GELU, fp32 I/O.

Sharding: data-parallel over batch B=8 -> one batch element per NeuronCore,
no collectives. Each core runs the full block on its [2048, 384] slice.

Design (v2):
  - Host ships x twice: token-major (xtok) and feature-major (xtr), both
    bf16.  No PE transposes anywhere in the kernel.
  - RMSNorm: sq = x^T * x^T (DVE), ms row via ones-matmul reduction (PE),
    sqrt on ACT + reciprocal_approx_fast (DVE), then a ones-outer-product
    matmul broadcasts the scale row to 128 partitions; h^T = x^T * s (DVE).
  - QK scores run as K=64 row-tiled matmul pairs: even head contracts
    array rows 0:63, odd head rows 64:127 (tile_position auto-derived from
    the kt/qt slices).  No zero-padded Q parity variants, and each head
    pays only its true 64-deep contraction.
  - Attention in S^T layout: exp on ScalarE with the causal handled by
    skipping dead tiles, suffix-exp + a 0/1 band multiply on diagonal
    tiles.  V is token-major with a ones column per head (slot width 65)
    so the AV matmul emits the softmax normalizer Z at PSUM row 64.
  - x2^T = x^T + wo^T o^T computed by matmul from the feature-major OT
    (no transposes); norm2 stats accumulate during attention; its sqrt
    batches at the attention->FFN boundary so ACT never switches tables
    mid-exp.
  - Output tile = one PSUM accumulation: I@x + o@wo + gelu@fw2 + b2
    (residuals included), so no token-major x2 is ever materialized.
  - ScalarE runs ONLY exp / gelu / sqrt; all copies go to DVE, the
    normalizer broadcast goes to GpSimd.
"""

import math
import sys

import ml_dtypes
import numpy as np

for _p in ("/opt/trn_rl_repo",):
    if _p not in sys.path:
        sys.path.append(_p)

import concourse.bacc as bacc
import concourse.bass as bass
import concourse.mybir as mybir
import concourse.tile as tile
from concourse.bass import ts
from concourse.bass_utils import run_bass_kernel_spmd
from concourse.masks import make_identity

F32 = mybir.dt.float32
BF16 = mybir.dt.bfloat16
AF = mybir.ActivationFunctionType

NCORES = 8
T, D, NH, HD, HDIM = 2048, 384, 6, 64, 768
P = 128
SLOT = HD + 1          # per-head V slot: [v_0..v_63, ones]
NT = T // P            # 16 token tiles
ND = D // P            # 3 feature chunks
NHT = HDIM // P        # 6 FFN hidden chunks
CH = 512               # q/token chunk width
NCH = T // CH          # 4
EPS = 1e-6
SCL = 1.0 / math.sqrt(HD)


def _body(tc, din, out_d):
    nc = tc.nc

    main_cm = tc.tile_pool(name="main", bufs=1)
    main = main_cm.__enter__()
    pscr_cm = tc.tile_pool(name="scr", bufs=3)
    pscr = pscr_cm.__enter__()
    patt_cm = tc.tile_pool(name="att", bufs=2)
    patt = patt_cm.__enter__()
    pout_cm = tc.tile_pool(name="outs", bufs=3)
    pout = pout_cm.__enter__()
    paux_cm = tc.tile_pool(name="paux", bufs=2, space="PSUM")
    paux = paux_cm.__enter__()

    def mt(shape, tag, dt_=BF16):
        return main.tile(shape, dt_, tag=tag, name=tag)

    # ---- constants ----
    ident = mt([P, P], "ident")
    make_identity(nc, ident[:])
    eps_t = mt([P, 1], "eps", F32)
    nc.gpsimd.memset(eps_t[:], EPS)
    onesf = mt([P, P], "onesf", F32)
    nc.gpsimd.memset(onesf[:], 1.0)
    ones_bf = mt([P, P], "onesbf")          # bf16 ones: col/row lhsT uses
    nc.vector.tensor_copy(ones_bf[:], onesf[:])
    band = mt([P, 896], "band", F32)
    nc.gpsimd.memset(band[:], 1.0)
    nc.gpsimd.affine_select(out=band[:], in_=band[:],
                            compare_op=mybir.AluOpType.is_ge,
                            fill=0.0, base=-384, channel_multiplier=-1,
                            pattern=[[1, 896]])

    # ---- input DMAs ----
    xtok = [mt([P, D], f"xtok{j}") for j in range(NT)]
    for j in range(NT):
        nc.sync.dma_start(xtok[j][:], din["xtok"][ts(j, P), :])
    xtr = [mt([P, T], f"xtr{c}") for c in range(ND)]
    for c in range(ND):
        nc.sync.dma_start(xtr[c][:], din["xtr"][ts(c, P), :])
    wq_s = [mt([P, D], f"wq{c}") for c in range(ND)]
    wk_s = [mt([P, D], f"wk{c}") for c in range(ND)]
    wv_s = [mt([P, D], f"wv{c}") for c in range(ND)]
    wo_s = [mt([P, D], f"wo{c}") for c in range(ND)]
    for c in range(ND):
        nc.sync.dma_start(wq_s[c][:], din["wq"][ts(c, P), :])
        nc.sync.dma_start(wk_s[c][:], din["wk"][ts(c, P), :])
        nc.sync.dma_start(wv_s[c][:], din["wv"][ts(c, P), :])
        nc.sync.dma_start(wo_s[c][:], din["wo"][ts(c, P), :])
    fw1_s = [mt([P, HDIM], f"fw1_{c}") for c in range(ND)]
    fw2_s = [mt([P, D], f"fw2_{c}") for c in range(NHT)]
    for c in range(ND):
        nc.sync.dma_start(fw1_s[c][:], din["fw1"][ts(c, P), :])
    for c in range(NHT):
        nc.sync.dma_start(fw2_s[c][:], din["fw2"][ts(c, P), :])
    b1_s = mt([P, NHT], "b1", F32)
    b2_row = mt([1, D], "b2")
    nc.sync.dma_start(b1_s[:], din["fb1"].rearrange("(a b) -> b a", b=P))
    nc.sync.dma_start(b2_row[:], din["fb2"].rearrange("(a b) -> a b", a=1))

    # ---- rmsnorm helper: produce s_bcast [P, T] bf16 from sq-tiles ----
    def norm_scale_bcast(src_tiles, ms_row, s_row, s_bf_row, s_bcast):
        # squares + ms row (per 512-chunk ones-matmul reduction)
        for ch in range(NCH):
            sqs = []
            for c in range(ND):
                t = pscr.tile([P, CH], BF16, tag="nsq", name=f"nsq{c}")
                nc.vector.tensor_mul(t[:], src_tiles[c][:, ts(ch, CH)],
                                     src_tiles[c][:, ts(ch, CH)])
                sqs.append(t)
            ms = paux.tile([P, CH], F32, tag="aux", name="ms")
            for c in range(ND):
                nc.tensor.matmul(ms[0:1, :], ones_bf[:, 0:1], sqs[c][:],
                                 start=(c == 0), stop=(c == ND - 1))
            nc.vector.tensor_copy(ms_row[0:1, ts(ch, CH)], ms[0:1, :])
        # rms = sqrt(ms/D + eps); s = 1/rms
        nc.scalar.activation(s_row[0:1, :], ms_row[0:1, :], AF.Sqrt,
                             scale=1.0 / D, bias=eps_t[0:1, 0:1])
        nc.vector.reciprocal_approx_fast(s_row[0:1, :], s_row[0:1, :])
        nc.vector.tensor_copy(s_bf_row[0:1, :], s_row[0:1, :])
        for ch in range(NCH):
            bb = paux.tile([P, CH], F32, tag="aux", name="bb")
            nc.tensor.matmul(bb[:], ones_bf[0:1, :],
                             s_bf_row[0:1, ts(ch, CH)], start=True, stop=True)
            nc.vector.tensor_copy(s_bcast[:, ts(ch, CH)], bb[:])

    # ---- norm1 -> HT ----
    ms1_row = mt([1, T], "ms1", F32)
    s1_row = mt([1, T], "s1r", F32)
    s1_bf = mt([1, T], "s1bf")
    s1b = mt([P, T], "s1b")
    norm_scale_bcast(xtr, ms1_row, s1_row, s1_bf, s1b)
    ht = [mt([P, T], f"ht{c}") for c in range(ND)]
    for c in range(ND):
        nc.vector.tensor_mul(ht[c][:], xtr[c][:], s1b[:])

    # ---- projections: K^T, Q^T (feature-major), V_aug (token-major) ----
    kt = [mt([P, T], f"kt{c}") for c in range(ND)]
    qt = [mt([P, T], f"qt{c}") for c in range(ND)]
    for dst, w_s in ((kt, wk_s), (qt, wq_s)):
        for dt in range(ND):
            for ch in range(NCH):
                ps = paux.tile([P, CH], F32, tag="aux", name="proj")
                for c in range(ND):
                    nc.tensor.matmul(ps[:], w_s[c][:, ts(dt, P)],
                                     ht[c][:, ts(ch, CH)],
                                     start=(c == 0), stop=(c == ND - 1))
                nc.vector.tensor_copy(dst[dt][:, ts(ch, CH)], ps[:])

    vaug = [mt([P, NH * SLOT], f"va{j}") for j in range(NT)]
    for j in range(NT):
        nc.vector.tensor_copy(
            vaug[j][:].rearrange("p (h e) -> p h e", h=NH)[:, :, HD : SLOT],
            onesf[:, 0:NH].rearrange("p (h e) -> p h e", e=1),
        )
        ps = paux.tile([P, CH], F32, tag="aux", name="vproj")
        for c in range(ND):
            nc.tensor.matmul(ps[:, 0:D], ht[c][:, ts(j, P)], wv_s[c][:],
                             start=(c == 0), stop=(c == ND - 1))
        nc.vector.tensor_copy(
            vaug[j][:].rearrange("p (h e) -> p h e", h=NH)[:, :, 0 : HD],
            ps[:, 0:D].rearrange("p (h e) -> p h e", h=NH),
        )

    # ---- attention + x2^T (norm2 stats accumulate along the way) ----
    ot = [mt([P, T], f"ot{c}") for c in range(ND)]
    x2t = [mt([P, T], f"x2t{c}") for c in range(ND)]
    ms2_row = mt([1, T], "ms2", F32)

    psS_cm = tc.tile_pool(name="psS", bufs=1, space="PSUM")
    psS = psS_cm.__enter__()
    psO_cm = tc.tile_pool(name="psO", bufs=1, space="PSUM")
    psO = psO_cm.__enter__()

    def exp_tile(p_sb, s_ps, ch, kt0):
        """exp over an [P, 2CH] score pair with causal masking."""
        d1 = (kt0 + 1) * P - ch * CH
        if d1 < 0:
            nc.scalar.activation(p_sb[:], s_ps[:], AF.Exp, scale=SCL)
            return
        for m in range(2):
            d = (kt0 + m) * P - ch * CH
            if d < 0:
                nc.scalar.activation(p_sb[:, ts(m, CH)], s_ps[:, ts(m, CH)],
                                     AF.Exp, scale=SCL)
            else:
                w = CH - d
                if d > 0:
                    nc.gpsimd.memset(p_sb[:, m * CH : m * CH + d], 0.0)
                p_f = patt.tile([P, CH], F32, tag="pf", name="pf")
                nc.scalar.activation(p_f[:, 0:w],
                                     s_ps[:, m * CH + d : (m + 1) * CH],
                                     AF.Exp, scale=SCL)
                nc.vector.tensor_mul(p_sb[:, m * CH + d : (m + 1) * CH],
                                     p_f[:, 0:w], band[:, 384 : 896 - d])

    for ch in range(NCH):
        ntk = 4 * (ch + 1)
        sl = ts(ch, CH)
        for dt in range(ND):
            h_e, h_o = 2 * dt, 2 * dt + 1
            o_e = psO.tile([P, CH], F32, tag="oe", name="oe")
            o_o = psO.tile([P, CH], F32, tag="oo", name="oo")
            for kt0 in range(0, ntk, 2):
                s_e = psS.tile([P, 2 * CH], F32, tag="se", name="se")
                s_o = psS.tile([P, 2 * CH], F32, tag="so", name="so")
                for m in range(2):
                    k = kt0 + m
                    nc.tensor.matmul(s_e[:, ts(m, CH)],
                                     kt[dt][0:HD, ts(k, P)],
                                     qt[dt][0:HD, sl], start=True, stop=True)
                    nc.tensor.matmul(s_o[:, ts(m, CH)],
                                     kt[dt][HD:P, ts(k, P)],
                                     qt[dt][HD:P, sl], start=True, stop=True)
                p_e = patt.tile([P, 2 * CH], BF16, tag="pe", name="pe")
                p_o = patt.tile([P, 2 * CH], BF16, tag="po", name="po")
                exp_tile(p_e, s_e, ch, kt0)
                exp_tile(p_o, s_o, ch, kt0)
                for m in range(2):
                    k = kt0 + m
                    nc.tensor.matmul(
                        o_e[0:SLOT, :],
                        vaug[k][:, h_e * SLOT : (h_e + 1) * SLOT],
                        p_e[:, ts(m, CH)],
                        start=(k == 0), stop=(k == ntk - 1))
                    nc.tensor.matmul(
                        o_o[0:SLOT, :],
                        vaug[k][:, h_o * SLOT : (h_o + 1) * SLOT],
                        p_o[:, ts(m, CH)],
                        start=(k == 0), stop=(k == ntk - 1))
            # normalize: Z at psum row 64 -> sbuf row 0, recip, broadcast
            for o_ps, lo, hi in ((o_e, 0, HD), (o_o, HD, P)):
                rz = patt.tile([P, CH], F32, tag="rz", name="rz")
                nc.vector.tensor_copy(rz[0:1, :], o_ps[HD : HD + 1, :])
                nc.vector.reciprocal_approx_fast(rz[0:1, :], rz[0:1, :])
                rzb = patt.tile([P, CH], F32, tag="rzb", name="rzb")
                nc.gpsimd.partition_broadcast(rzb[0:HD, :], rz[0:1, :])
                nc.vector.tensor_mul(ot[dt][lo:hi, sl], o_ps[0:HD, :],
                                     rzb[0:HD, :])
        # x2^T chunk = x^T + wo^T @ o^T ; then norm2 stats for this chunk
        sqs = []
        for dt2 in range(ND):
            xx = paux.tile([P, CH], F32, tag="aux", name="xx")
            for c in range(ND):
                nc.tensor.matmul(xx[:], wo_s[c][:, ts(dt2, P)],
                                 ot[c][:, sl],
                                 start=(c == 0), stop=(c == ND - 1))
            nc.vector.tensor_add(x2t[dt2][:, sl], xx[:], xtr[dt2][:, sl])
            sq = pscr.tile([P, CH], BF16, tag="sq2", name="sq2")
            nc.vector.tensor_mul(sq[:], x2t[dt2][:, sl], x2t[dt2][:, sl])
            sqs.append(sq)
        ms2 = paux.tile([P, CH], F32, tag="aux", name="ms2")
        for dt2 in range(ND):
            nc.tensor.matmul(ms2[0:1, :], ones_bf[:, 0:1], sqs[dt2][:],
                             start=(dt2 == 0), stop=(dt2 == ND - 1))
        nc.vector.tensor_copy(ms2_row[0:1, sl], ms2[0:1, :])

    psO_cm.__exit__(None, None, None)
    psS_cm.__exit__(None, None, None)

    # ---- norm2 scale (batched: single table switch) + H2T ----
    s2_row = mt([1, T], "s2r", F32)
    s2_bf = mt([1, T], "s2bf")
    s2b = mt([P, T], "s2b")
    nc.scalar.activation(s2_row[0:1, :], ms2_row[0:1, :], AF.Sqrt,
                         scale=1.0 / D, bias=eps_t[0:1, 0:1])
    nc.vector.reciprocal_approx_fast(s2_row[0:1, :], s2_row[0:1, :])
    nc.vector.tensor_copy(s2_bf[0:1, :], s2_row[0:1, :])
    psF_cm = tc.tile_pool(name="psF", bufs=2, space="PSUM")
    psF = psF_cm.__enter__()
    psG_cm = tc.tile_pool(name="psG", bufs=2, space="PSUM")
    psG = psG_cm.__enter__()
    for ch in range(NCH):
        bb = paux.tile([P, CH], F32, tag="aux", name="bb2")
        nc.tensor.matmul(bb[:], ones_bf[0:1, :], s2_bf[0:1, ts(ch, CH)],
                         start=True, stop=True)
        nc.vector.tensor_copy(s2b[:, ts(ch, CH)], bb[:])
    # h2t reuses the (dead) ht slots
    h2t = [main.tile([P, T], BF16, tag=f"ht{c}", name=f"h2t{c}")
           for c in range(ND)]
    for c in range(ND):
        nc.vector.tensor_mul(h2t[c][:], x2t[c][:], s2b[:])

    # ---- FFN hidden + GELU (gt reuses qt/kt slots) ----
    gt = [main.tile([P, T], BF16, tag=(f"qt{c}" if c < ND else f"kt{c - ND}"),
                    name=f"gt{c}") for c in range(NHT)]
    for half in range(NCH // 2):
        for htile in range(NHT):
            ps = psF.tile([P, 2 * CH], F32, tag="a1", name="a1")
            for m in range(2):
                for c in range(ND):
                    nc.tensor.matmul(ps[:, ts(m, CH)],
                                     fw1_s[c][:, ts(htile, P)],
                                     h2t[c][:, ts(2 * half + m, CH)],
                                     start=(c == 0), stop=(c == ND - 1))
            nc.scalar.activation(gt[htile][:, ts(half, 2 * CH)], ps[:],
                                 AF.Gelu, bias=b1_s[:, htile : htile + 1])

        # ---- output tiles for this half: x + o@wo + ffn + b2 ----
        for j in range(8 * half, 8 * (half + 1)):
            ps = psG.tile([P, D], F32, tag="g", name="g")
            nc.tensor.matmul(ps[:], ident[:], xtok[j][:],
                             start=True, stop=False)
            for c in range(ND):
                nc.tensor.matmul(ps[:], ot[c][:, ts(j, P)], wo_s[c][:],
                                 start=False, stop=False)
            for c in range(NHT):
                nc.tensor.matmul(ps[:], gt[c][:, ts(j, P)], fw2_s[c][:],
                                 start=False, stop=False)
            nc.tensor.matmul(ps[:], ones_bf[0:1, :], b2_row[0:1, :],
                             start=False, stop=True)
            o_t = pout.tile([P, D], F32, tag="o", name="o")
            nc.vector.tensor_copy(o_t[:], ps[:])
            nc.sync.dma_start(out_d[ts(j, P), :], o_t[:])

    psG_cm.__exit__(None, None, None)
    psF_cm.__exit__(None, None, None)
    paux_cm.__exit__(None, None, None)
    pout_cm.__exit__(None, None, None)
    patt_cm.__exit__(None, None, None)
    pscr_cm.__exit__(None, None, None)
    main_cm.__exit__(None, None, None)


_CACHE = {}


def _build():
    if "nc" in _CACHE:
        return _CACHE["nc"]
    nc = bacc.Bacc("TRN2", target_bir_lowering=False, debug=False)
    din = {}
    for name, shape, dt_ in (
        ("xtok", [T, D], BF16), ("xtr", [D, T], BF16),
        ("wq", [D, D], BF16), ("wk", [D, D], BF16),
        ("wv", [D, D], BF16), ("wo", [D, D], BF16),
        ("fw1", [D, HDIM], BF16), ("fb1", [HDIM], F32),
        ("fw2", [HDIM, D], BF16), ("fb2", [D], BF16),
    ):
        din[name] = nc.dram_tensor(name, shape, dt_, kind="ExternalInput").ap()
    out_d = nc.dram_tensor("out", [T, D], F32, kind="ExternalOutput").ap()
    with tile.TileContext(nc) as tc:
        _body(tc, din, out_d)
    nc.compile()
    _CACHE["nc"] = nc
    return nc


def run(inputs: dict, trace: bool = False):
    """Run on 8 cores; returns (output [8,T,D], BassKernelResults)."""
    nc = _build()
    x = np.asarray(inputs["x"], dtype=np.float32)
    ln1 = np.asarray(inputs["ln1_w"], dtype=np.float32)
    ln2 = np.asarray(inputs["ln2_w"], dtype=np.float32)
    bf = ml_dtypes.bfloat16
    shared = {
        "wq": (ln1[:, None] * np.asarray(inputs["wq"], np.float32)).astype(bf),
        "wk": (ln1[:, None] * np.asarray(inputs["wk"], np.float32)).astype(bf),
        "wv": (ln1[:, None] * np.asarray(inputs["wv"], np.float32)).astype(bf),
        "wo": np.asarray(inputs["wo"], np.float32).astype(bf),
        "fw1": (ln2[:, None] * np.asarray(inputs["ff_w1"], np.float32)).astype(bf),
        "fb1": np.asarray(inputs["ff_b1"], np.float32),
        "fw2": np.asarray(inputs["ff_w2"], np.float32).astype(bf),
        "fb2": np.asarray(inputs["ff_b2"], np.float32).astype(bf),
    }
    shared = {k: np.ascontiguousarray(v) for k, v in shared.items()}
    in_maps = [
        dict(shared,
             xtok=np.ascontiguousarray(x[c].astype(bf)),
             xtr=np.ascontiguousarray(x[c].T.astype(bf)))
        for c in range(NCORES)
    ]
    res = run_bass_kernel_spmd(nc, in_maps, list(range(NCORES)), trace=trace)
    out = np.stack([res.results[c]["out"] for c in range(NCORES)], axis=0)
    return out, res


def kernel(**inputs) -> np.ndarray:
    out, _ = run(inputs, trace=False)
    return out


# revision 12
# speedup vs baseline: 1.0415x; 1.0415x over previous
"""Trainium2 Bass kernel for a dense transformer block.

Problem: B=8, T=2048, DIM=384, 6 heads (hd=64), FFN hidden 768, causal
attention, RMSNorm (eps 1e-6), exact GELU, fp32 I/O.

Sharding: data-parallel over batch B=8 -> one batch element per NeuronCore,
no collectives. Each core runs the full block on its [2048, 384] slice.

Design (v3):
  - Host ships x twice: token-major (xtok) and feature-major (xtr), both
    bf16.  No PE transposes anywhere in the kernel.  All inputs arrive in
    12 batched DMAs (multi-tile rearranged access patterns), critical
    tensors first.
  - RMSNorm: sq = x^T*x^T (DVE), ms row via ones-matmul reduction (PE),
    sqrt row on ACT + reciprocal_approx_fast (DVE), ones-outer-product
    matmul broadcast, h^T = x^T * s (DVE).
  - QK scores as K=64 row-tiled matmul pairs (even head rows 0:63, odd
    head 64:127; tile_position auto-derived).  Attention emission is
    software-pipelined: per stage [exp_e, QK_e(next), exp_o, QK_o(next),
    AV_e, AV_o] so ScalarE never waits on a head-of-line blocked QK.
  - V token-major with a ones column per head (slot 65) -> softmax Z free
    at PSUM row 64; normalize = recip_approx + gpsimd partition_broadcast,
    writing the feature-major OT directly (cross-partition DVE mul).
  - x2^T = x^T + wo^T o^T by matmul per chunk during attention; norm2
    stats (squares + ones-matmul) also per chunk during attention; the
    single norm2 sqrt batches at the attention->FFN boundary so ACT never
    switches tables mid-exp.
  - Tail: per-chunk scale broadcast + h2^T muls, FFN1+GELU, then output
    accumulation o@wo + gelu@fw2 + b2 in PSUM; the x residual is added by
    the DVE on the way out.  projection copies run on the otherwise-idle
    ScalarE during the projection phase.
"""

import math
import sys

import ml_dtypes
import numpy as np

for _p in ("/opt/trn_rl_repo",):
    if _p not in sys.path:
        sys.path.append(_p)

import concourse.bacc as bacc
import concourse.bass as bass
import concourse.mybir as mybir
import concourse.tile as tile
from concourse.bass import ts
from concourse.bass_utils import run_bass_kernel_spmd
from concourse.masks import make_identity

F32 = mybir.dt.float32
BF16 = mybir.dt.bfloat16
AF = mybir.ActivationFunctionType

NCORES = 8
T, D, NH, HD, HDIM = 2048, 384, 6, 64, 768
P = 128
SLOT = HD + 1          # per-head V slot: [v_0..v_63, ones]
NT = T // P            # 16 token tiles
ND = D // P            # 3 feature chunks
NHT = HDIM // P        # 6 FFN hidden chunks
CH = 512               # q/token chunk width
NCH = T // CH          # 4
EPS = 1e-6
SCL = 1.0 / math.sqrt(HD)


def _body(tc, din, out_d):
    nc = tc.nc

    main_cm = tc.tile_pool(name="main", bufs=1)
    main = main_cm.__enter__()
    pscr_cm = tc.tile_pool(name="scr", bufs=3)
    pscr = pscr_cm.__enter__()
    patt_cm = tc.tile_pool(name="att", bufs=3)
    patt = patt_cm.__enter__()
    pnrm_cm = tc.tile_pool(name="nrm", bufs=2)
    pnrm = pnrm_cm.__enter__()
    pout_cm = tc.tile_pool(name="outs", bufs=3)
    pout = pout_cm.__enter__()
    paux_cm = tc.tile_pool(name="paux", bufs=2, space="PSUM")
    paux = paux_cm.__enter__()

    def mt(shape, tag, dt_=BF16):
        return main.tile(shape, dt_, tag=tag, name=tag)

    # ---- input DMAs (batched; critical tensors first) ----
    xtr = [mt([P, T], f"xtr{c}") for c in range(ND)]
    for c in range(ND):
        nc.sync.dma_start(xtr[c][:], din["xtr"][ts(c, P), :])
    wk_a = mt([P, ND * D], "wka")
    wq_a = mt([P, ND * D], "wqa")
    wv_a = mt([P, ND * D], "wva")
    wo_a = mt([P, ND * D], "woa")
    for dst, name in ((wk_a, "wk"), (wq_a, "wq"), (wv_a, "wv"), (wo_a, "wo")):
        nc.sync.dma_start(dst[:].rearrange("p (c d) -> p c d", d=D),
                          din[name].rearrange("(c p) d -> p c d", p=P))
    fw1_a = mt([P, ND * HDIM], "fw1a")
    nc.sync.dma_start(fw1_a[:].rearrange("p (c d) -> p c d", d=HDIM),
                      din["fw1"].rearrange("(c p) d -> p c d", p=P))
    fw2_a = mt([P, NHT * D], "fw2a")
    nc.sync.dma_start(fw2_a[:].rearrange("p (c d) -> p c d", d=D),
                      din["fw2"].rearrange("(c p) d -> p c d", p=P))
    b1_s = mt([P, NHT], "b1", F32)
    b2_row = mt([1, D], "b2")
    nc.sync.dma_start(b1_s[:], din["fb1"].rearrange("(a b) -> b a", b=P))
    nc.sync.dma_start(b2_row[:], din["fb2"].rearrange("(a b) -> a b", a=1))
    xtok_a = mt([P, NT * D], "xtoka")
    nc.sync.dma_start(xtok_a[:].rearrange("p (j d) -> p j d", d=D),
                      din["xtok"].rearrange("(j p) d -> p j d", p=P))

    wk_s = [wk_a[:, ts(c, D)] for c in range(ND)]
    wq_s = [wq_a[:, ts(c, D)] for c in range(ND)]
    wv_s = [wv_a[:, ts(c, D)] for c in range(ND)]
    wo_s = [wo_a[:, ts(c, D)] for c in range(ND)]
    fw1_s = [fw1_a[:, ts(c, HDIM)] for c in range(ND)]
    fw2_s = [fw2_a[:, ts(c, D)] for c in range(NHT)]
    xtok = [xtok_a[:, ts(j, D)] for j in range(NT)]

    # ---- constants ----
    eps_t = mt([P, 1], "eps", F32)
    nc.gpsimd.memset(eps_t[:], EPS)
    onesf = mt([P, P], "onesf", F32)
    nc.gpsimd.memset(onesf[:], 1.0)
    ones_bf = mt([P, P], "onesbf")
    nc.vector.tensor_copy(ones_bf[:], onesf[:])
    band = mt([P, 896], "band", F32)
    nc.gpsimd.memset(band[:], 1.0)
    nc.gpsimd.affine_select(out=band[:], in_=band[:],
                            compare_op=mybir.AluOpType.is_ge,
                            fill=0.0, base=-384, channel_multiplier=-1,
                            pattern=[[1, 896]])

    # ---- rmsnorm scale helper (squares assumed done): ms rows -> s_bcast --
    def norm_rows(src_tiles, ms_row):
        for ch in range(NCH):
            sqs = []
            for c in range(ND):
                t = pscr.tile([P, CH], BF16, tag="nsq", name=f"nsq{c}")
                nc.vector.tensor_mul(t[:], src_tiles[c][:, ts(ch, CH)],
                                     src_tiles[c][:, ts(ch, CH)])
                sqs.append(t)
            ms = paux.tile([P, CH], F32, tag="aux", name="ms")
            for c in range(ND):
                nc.tensor.matmul(ms[0:1, :], ones_bf[:, 0:1], sqs[c][:],
                                 start=(c == 0), stop=(c == ND - 1))
            nc.vector.tensor_copy(ms_row[0:1, ts(ch, CH)], ms[0:1, :])

    def scale_row(ms_row, s_row, s_bf_row):
        # rms = sqrt(ms/D + eps); s = 1/rms
        nc.scalar.activation(s_row[0:1, :], ms_row[0:1, :], AF.Sqrt,
                             scale=1.0 / D, bias=eps_t[0:1, 0:1])
        nc.vector.reciprocal_approx_fast(s_row[0:1, :], s_row[0:1, :])
        nc.vector.tensor_copy(s_bf_row[0:1, :], s_row[0:1, :])

    def bcast_chunk(s_bf_row, s_bcast, ch):
        bb = paux.tile([P, CH], F32, tag="aux", name="bb")
        nc.tensor.matmul(bb[:], ones_bf[0:1, :], s_bf_row[0:1, ts(ch, CH)],
                         start=True, stop=True)
        nc.vector.tensor_copy(s_bcast[:, ts(ch, CH)], bb[:])

    # ---- norm1 -> HT ----
    ms1_row = mt([1, T], "ms1", F32)
    s1_row = mt([1, T], "s1r", F32)
    s1_bf = mt([1, T], "s1bf")
    s1b = mt([P, T], "s1b")
    norm_rows(xtr, ms1_row)
    scale_row(ms1_row, s1_row, s1_bf)
    # preload the exp table set while ACT is otherwise idle
    dummy = mt([1, 1], "dummy", F32)
    nc.scalar.activation(dummy[0:1, :], eps_t[0:1, 0:1], AF.Exp)
    ht = [mt([P, T], f"ht{c}") for c in range(ND)]
    for ch in range(NCH):
        bcast_chunk(s1_bf, s1b, ch)
    for c in range(ND):
        nc.vector.tensor_mul(ht[c][:], xtr[c][:], s1b[:])

    # ---- projections: K^T, Q^T (feature-major), V_aug (token-major) ----
    # psum->sbuf copies run on ScalarE (idle during this phase).
    kt = [mt([P, T], f"kt{c}") for c in range(ND)]
    qt = [mt([P, T], f"qt{c}") for c in range(ND)]
    for ch in range(NCH):
        for dst, w_s in ((kt, wk_s), (qt, wq_s)):
            for dt in range(ND):
                ps = paux.tile([P, CH], F32, tag="aux", name="proj")
                for c in range(ND):
                    nc.tensor.matmul(ps[:], w_s[c][:, ts(dt, P)],
                                     ht[c][:, ts(ch, CH)],
                                     start=(c == 0), stop=(c == ND - 1))
                nc.scalar.copy(dst[dt][:, ts(ch, CH)], ps[:])

    vaug = [mt([P, NH * SLOT], f"va{j}") for j in range(NT)]
    for j in range(NT):
        nc.vector.tensor_copy(
            vaug[j][:].rearrange("p (h e) -> p h e", h=NH)[:, :, HD : SLOT],
            onesf[:, 0:NH].rearrange("p (h e) -> p h e", e=1),
        )
        ps = paux.tile([P, CH], F32, tag="aux", name="vproj")
        for c in range(ND):
            nc.tensor.matmul(ps[:, 0:D], ht[c][:, ts(j, P)], wv_s[c][:],
                             start=(c == 0), stop=(c == ND - 1))
        nc.scalar.copy(
            vaug[j][:].rearrange("p (h e) -> p h e", h=NH)[:, :, 0 : HD],
            ps[:, 0:D].rearrange("p (h e) -> p h e", h=NH),
        )

    # ---- attention + x2^T + norm2 stats ----
    ot = [mt([P, T], f"ot{c}") for c in range(ND)]
    x2t = [mt([P, T], f"x2t{c}") for c in range(ND)]
    ms2_row = mt([1, T], "ms2", F32)

    psS_cm = tc.tile_pool(name="psS", bufs=1, space="PSUM")
    psS = psS_cm.__enter__()
    psO_cm = tc.tile_pool(name="psO", bufs=1, space="PSUM")
    psO = psO_cm.__enter__()

    def exp_tile(p_sb, s_ps, ch, kt0):
        """exp over an [P, 2CH] score pair with causal masking."""
        d1 = (kt0 + 1) * P - ch * CH
        if d1 < 0:
            nc.scalar.activation(p_sb[:], s_ps[:], AF.Exp, scale=SCL)
            return
        for m in range(2):
            d = (kt0 + m) * P - ch * CH
            if d < 0:
                nc.scalar.activation(p_sb[:, ts(m, CH)], s_ps[:, ts(m, CH)],
                                     AF.Exp, scale=SCL)
            else:
                w = CH - d
                if d > 0:
                    nc.gpsimd.memset(p_sb[:, m * CH : m * CH + d], 0.0)
                p_f = pnrm.tile([P, CH], F32, tag="pf", name="pf")
                nc.scalar.activation(p_f[:, 0:w],
                                     s_ps[:, m * CH + d : (m + 1) * CH],
                                     AF.Exp, scale=SCL)
                nc.vector.tensor_mul(p_sb[:, m * CH + d : (m + 1) * CH],
                                     p_f[:, 0:w], band[:, 384 : 896 - d])

    def qk_pair(st, ch, par):
        """Emit the K=64 row-tiled QK matmuls for one head of stage st."""
        dt, kt0 = st
        lo, hi = (0, HD) if par == 0 else (HD, P)
        s_ps = psS.tile([P, 2 * CH], F32, tag=("se" if par == 0 else "so"),
                        name="s")
        for m in range(2):
            nc.tensor.matmul(s_ps[:, ts(m, CH)],
                             kt[dt][lo:hi, ts(kt0 + m, P)],
                             qt[dt][lo:hi, ts(ch, CH)], start=True, stop=True)
        return s_ps

    for ch in range(NCH):
        ntk = 4 * (ch + 1)
        sl = ts(ch, CH)
        stages = [(dt, kt0) for dt in range(ND) for kt0 in range(0, ntk, 2)]
        o_ps = {}
        s_cur = {}
        s_cur[0] = qk_pair(stages[0], ch, 0)
        s_cur[1] = qk_pair(stages[0], ch, 1)
        for i, (dt, kt0) in enumerate(stages):
            if kt0 == 0:
                o_ps[(dt, 0)] = psO.tile([P, CH], F32, tag="oe", name="oe")
                o_ps[(dt, 1)] = psO.tile([P, CH], F32, tag="oo", name="oo")
            nxt = stages[i + 1] if i + 1 < len(stages) else None
            p_sb = {}
            s_stage = (s_cur[0], s_cur[1])
            for par in range(2):
                p_sb[par] = patt.tile([P, 2 * CH], BF16,
                                      tag=("pe" if par == 0 else "po"),
                                      name="p")
                exp_tile(p_sb[par], s_stage[par], ch, kt0)
                if nxt is not None:
                    s_cur[par] = qk_pair(nxt, ch, par)
            for par in range(2):
                h = 2 * dt + par
                for m in range(2):
                    k = kt0 + m
                    nc.tensor.matmul(
                        o_ps[(dt, par)][0:SLOT, :],
                        vaug[k][:, h * SLOT : (h + 1) * SLOT],
                        p_sb[par][:, ts(m, CH)],
                        start=(k == 0), stop=(k == ntk - 1))
            if kt0 + 2 >= ntk:
                # last k-pair of this head pair: normalize both heads
                for par in range(2):
                    lo, hi = (0, HD) if par == 0 else (HD, P)
                    rz = pnrm.tile([P, CH], F32, tag="rz", name="rz")
                    nc.vector.tensor_copy(rz[0:1, :],
                                          o_ps[(dt, par)][HD : HD + 1, :])
                    nc.vector.reciprocal_approx_fast(rz[0:1, :], rz[0:1, :])
                    rzb = pnrm.tile([P, CH], F32, tag="rzb", name="rzb")
                    nc.gpsimd.partition_broadcast(rzb[0:HD, :], rz[0:1, :])
                    nc.vector.tensor_mul(ot[dt][lo:hi, sl],
                                         o_ps[(dt, par)][0:HD, :],
                                         rzb[0:HD, :])
        # x2^T chunk = x^T + wo^T @ o^T ; norm2 stats for this chunk
        sqs = []
        for dt2 in range(ND):
            xx = paux.tile([P, CH], F32, tag="aux", name="xx")
            for c in range(ND):
                nc.tensor.matmul(xx[:], wo_s[c][:, ts(dt2, P)], ot[c][:, sl],
                                 start=(c == 0), stop=(c == ND - 1))
            nc.vector.tensor_add(x2t[dt2][:, sl], xx[:], xtr[dt2][:, sl])
            sq = pscr.tile([P, CH], BF16, tag="sq2", name="sq2")
            nc.vector.tensor_mul(sq[:], x2t[dt2][:, sl], x2t[dt2][:, sl])
            sqs.append(sq)
        ms2 = paux.tile([P, CH], F32, tag="aux", name="ms2")
        for dt2 in range(ND):
            nc.tensor.matmul(ms2[0:1, :], ones_bf[:, 0:1], sqs[dt2][:],
                             start=(dt2 == 0), stop=(dt2 == ND - 1))
        nc.vector.tensor_copy(ms2_row[0:1, sl], ms2[0:1, :])

    psO_cm.__exit__(None, None, None)
    psS_cm.__exit__(None, None, None)

    # ---- norm2 scale (batched: single table switch) ----
    s2_row = mt([1, T], "s2r", F32)
    s2_bf = mt([1, T], "s2bf")
    s2b = mt([P, T], "s2b")
    scale_row(ms2_row, s2_row, s2_bf)

    psF_cm = tc.tile_pool(name="psF", bufs=2, space="PSUM")
    psF = psF_cm.__enter__()
    psG_cm = tc.tile_pool(name="psG", bufs=2, space="PSUM")
    psG = psG_cm.__enter__()

    # h2t reuses the (dead) ht slots
    h2t = [main.tile([P, T], BF16, tag=f"ht{c}", name=f"h2t{c}")
           for c in range(ND)]
    for ch in range(NCH):
        bcast_chunk(s2_bf, s2b, ch)
        for c in range(ND):
            nc.vector.tensor_mul(h2t[c][:, ts(ch, CH)], x2t[c][:, ts(ch, CH)],
                                 s2b[:, ts(ch, CH)])

    # ---- FFN + output (gt reuses qt/kt slots) ----
    gt = [main.tile([P, T], BF16, tag=(f"qt{c}" if c < ND else f"kt{c - ND}"),
                    name=f"gt{c}") for c in range(NHT)]
    for half in range(NCH // 2):
        for htile in range(NHT):
            ps = psF.tile([P, 2 * CH], F32, tag="a1", name="a1")
            for m in range(2):
                for c in range(ND):
                    nc.tensor.matmul(ps[:, ts(m, CH)],
                                     fw1_s[c][:, ts(htile, P)],
                                     h2t[c][:, ts(2 * half + m, CH)],
                                     start=(c == 0), stop=(c == ND - 1))
            nc.scalar.activation(gt[htile][:, ts(half, 2 * CH)], ps[:],
                                 AF.Gelu, bias=b1_s[:, htile : htile + 1])

        for j in range(8 * half, 8 * (half + 1)):
            ps = psG.tile([P, D], F32, tag="g", name="g")
            for c in range(ND):
                nc.tensor.matmul(ps[:], ot[c][:, ts(j, P)], wo_s[c][:],
                                 start=(c == 0), stop=False)
            for c in range(NHT):
                nc.tensor.matmul(ps[:], gt[c][:, ts(j, P)], fw2_s[c][:],
                                 start=False, stop=False)
            nc.tensor.matmul(ps[:], ones_bf[0:1, :], b2_row[0:1, :],
                             start=False, stop=True)
            o_t = pout.tile([P, D], F32, tag="o", name="o")
            nc.vector.tensor_add(o_t[:], ps[:], xtok[j])
            nc.sync.dma_start(out_d[ts(j, P), :], o_t[:])

    psG_cm.__exit__(None, None, None)
    psF_cm.__exit__(None, None, None)
    paux_cm.__exit__(None, None, None)
    pout_cm.__exit__(None, None, None)
    pnrm_cm.__exit__(None, None, None)
    patt_cm.__exit__(None, None, None)
    pscr_cm.__exit__(None, None, None)
    main_cm.__exit__(None, None, None)


_CACHE = {}


def _build():
    if "nc" in _CACHE:
        return _CACHE["nc"]
    nc = bacc.Bacc("TRN2", target_bir_lowering=False, debug=False)
    din = {}
    for name, shape, dt_ in (
        ("xtok", [T, D], BF16), ("xtr", [D, T], BF16),
        ("wq", [D, D], BF16), ("wk", [D, D], BF16),
        ("wv", [D, D], BF16), ("wo", [D, D], BF16),
        ("fw1", [D, HDIM], BF16), ("fb1", [HDIM], F32),
        ("fw2", [HDIM, D], BF16), ("fb2", [D], BF16),
    ):
        din[name] = nc.dram_tensor(name, shape, dt_, kind="ExternalInput").ap()
    out_d = nc.dram_tensor("out", [T, D], F32, kind="ExternalOutput").ap()
    with tile.TileContext(nc) as tc:
        _body(tc, din, out_d)
    nc.compile()
    _CACHE["nc"] = nc
    return nc


def run(inputs: dict, trace: bool = False):
    """Run on 8 cores; returns (output [8,T,D], BassKernelResults)."""
    nc = _build()
    x = np.asarray(inputs["x"], dtype=np.float32)
    ln1 = np.asarray(inputs["ln1_w"], dtype=np.float32)
    ln2 = np.asarray(inputs["ln2_w"], dtype=np.float32)
    bf = ml_dtypes.bfloat16
    shared = {
        "wq": (ln1[:, None] * np.asarray(inputs["wq"], np.float32)).astype(bf),
        "wk": (ln1[:, None] * np.asarray(inputs["wk"], np.float32)).astype(bf),
        "wv": (ln1[:, None] * np.asarray(inputs["wv"], np.float32)).astype(bf),
        "wo": np.asarray(inputs["wo"], np.float32).astype(bf),
        "fw1": (ln2[:, None] * np.asarray(inputs["ff_w1"], np.float32)).astype(bf),
        "fb1": np.asarray(inputs["ff_b1"], np.float32),
        "fw2": np.asarray(inputs["ff_w2"], np.float32).astype(bf),
        "fb2": np.asarray(inputs["ff_b2"], np.float32).astype(bf),
    }
    shared = {k: np.ascontiguousarray(v) for k, v in shared.items()}
    in_maps = [
        dict(shared,
             xtok=np.ascontiguousarray(x[c].astype(bf)),
             xtr=np.ascontiguousarray(x[c].T.astype(bf)))
        for c in range(NCORES)
    ]
    res = run_bass_kernel_spmd(nc, in_maps, list(range(NCORES)), trace=trace)
    out = np.stack([res.results[c]["out"] for c in range(NCORES)], axis=0)
    return out, res


def kernel(**inputs) -> np.ndarray:
    out, _ = run(inputs, trace=False)
    return out


# revision 14
# speedup vs baseline: 1.0818x; 1.0387x over previous
"""Trainium2 Bass kernel for a dense transformer block.

Problem: B=8, T=2048, DIM=384, 6 heads (hd=64), FFN hidden 768, causal
attention, RMSNorm (eps 1e-6), exact GELU, fp32 I/O.

Sharding: data-parallel over batch B=8 -> one batch element per NeuronCore,
no collectives. Each core runs the full block on its [2048, 384] slice.

Design (v3):
  - Host ships x twice: token-major (xtok) and feature-major (xtr), both
    bf16.  No PE transposes anywhere in the kernel.  All inputs arrive in
    12 batched DMAs (multi-tile rearranged access patterns), critical
    tensors first.
  - RMSNorm: sq = x^T*x^T (DVE), ms row via ones-matmul reduction (PE),
    sqrt row on ACT + reciprocal_approx_fast (DVE), ones-outer-product
    matmul broadcast, h^T = x^T * s (DVE).
  - QK scores as K=64 row-tiled matmul pairs (even head rows 0:63, odd
    head 64:127; tile_position auto-derived).  Attention emission is
    software-pipelined: per stage [exp_e, QK_e(next), exp_o, QK_o(next),
    AV_e, AV_o] so ScalarE never waits on a head-of-line blocked QK.
  - V token-major with a ones column per head (slot 65) -> softmax Z free
    at PSUM row 64; normalize = recip_approx + gpsimd partition_broadcast,
    writing the feature-major OT directly (cross-partition DVE mul).
  - x2^T = x^T + wo^T o^T by matmul per chunk during attention; norm2
    stats (squares + ones-matmul) also per chunk during attention; the
    single norm2 sqrt batches at the attention->FFN boundary so ACT never
    switches tables mid-exp.
  - Tail: per-chunk scale broadcast + h2^T muls, FFN1+GELU, then output
    accumulation o@wo + gelu@fw2 + b2 in PSUM; the x residual is added by
    the DVE on the way out.  projection copies run on the otherwise-idle
    ScalarE during the projection phase.
"""

import math
import sys

import ml_dtypes
import numpy as np

for _p in ("/opt/trn_rl_repo",):
    if _p not in sys.path:
        sys.path.append(_p)

import concourse.bacc as bacc
import concourse.bass as bass
import concourse.mybir as mybir
import concourse.tile as tile
from concourse.bass import ts
from concourse.bass_utils import run_bass_kernel_spmd
from concourse.masks import make_identity

F32 = mybir.dt.float32
BF16 = mybir.dt.bfloat16
AF = mybir.ActivationFunctionType

NCORES = 8
T, D, NH, HD, HDIM = 2048, 384, 6, 64, 768
P = 128
SLOT = HD + 1          # per-head V slot: [v_0..v_63, ones]
NT = T // P            # 16 token tiles
ND = D // P            # 3 feature chunks
NHT = HDIM // P        # 6 FFN hidden chunks
CH = 512               # q/token chunk width
NCH = T // CH          # 4
EPS = 1e-6
SCL = 1.0 / math.sqrt(HD)


def _body(tc, din, out_d):
    nc = tc.nc

    main_cm = tc.tile_pool(name="main", bufs=1)
    main = main_cm.__enter__()
    pscr_cm = tc.tile_pool(name="scr", bufs=3)
    pscr = pscr_cm.__enter__()
    patt_cm = tc.tile_pool(name="att", bufs=3)
    patt = patt_cm.__enter__()
    pnrm_cm = tc.tile_pool(name="nrm", bufs=2)
    pnrm = pnrm_cm.__enter__()
    pout_cm = tc.tile_pool(name="outs", bufs=3)
    pout = pout_cm.__enter__()
    paux_cm = tc.tile_pool(name="paux", bufs=2, space="PSUM")
    paux = paux_cm.__enter__()

    def mt(shape, tag, dt_=BF16):
        return main.tile(shape, dt_, tag=tag, name=tag)

    # ---- input DMAs (batched; critical tensors first) ----
    xtr = [mt([P, T], f"xtr{c}") for c in range(ND)]
    for c in range(ND):
        nc.sync.dma_start(xtr[c][:], din["xtr"][ts(c, P), :])
    wk_a = mt([P, ND * D], "wka")
    wq_a = mt([P, ND * D], "wqa")
    wv_a = mt([P, ND * D], "wva")
    wo_a = mt([P, ND * D], "woa")
    for dst, name in ((wk_a, "wk"), (wq_a, "wq"), (wv_a, "wv"), (wo_a, "wo")):
        nc.sync.dma_start(dst[:].rearrange("p (c d) -> p c d", d=D),
                          din[name].rearrange("(c p) d -> p c d", p=P))
    fw1_a = mt([P, ND * HDIM], "fw1a")
    nc.sync.dma_start(fw1_a[:].rearrange("p (c d) -> p c d", d=HDIM),
                      din["fw1"].rearrange("(c p) d -> p c d", p=P))
    fw2_a = mt([P, NHT * D], "fw2a")
    nc.sync.dma_start(fw2_a[:].rearrange("p (c d) -> p c d", d=D),
                      din["fw2"].rearrange("(c p) d -> p c d", p=P))
    b1_s = mt([P, NHT], "b1", F32)
    b2_row = mt([1, D], "b2")
    nc.sync.dma_start(b1_s[:], din["fb1"].rearrange("(a b) -> b a", b=P))
    nc.sync.dma_start(b2_row[:], din["fb2"].rearrange("(a b) -> a b", a=1))
    xtok_a = mt([P, NT * D], "xtoka")
    nc.sync.dma_start(xtok_a[:].rearrange("p (j d) -> p j d", d=D),
                      din["xtok"].rearrange("(j p) d -> p j d", p=P))

    wk_s = [wk_a[:, ts(c, D)] for c in range(ND)]
    wq_s = [wq_a[:, ts(c, D)] for c in range(ND)]
    wv_s = [wv_a[:, ts(c, D)] for c in range(ND)]
    wo_s = [wo_a[:, ts(c, D)] for c in range(ND)]
    fw1_s = [fw1_a[:, ts(c, HDIM)] for c in range(ND)]
    fw2_s = [fw2_a[:, ts(c, D)] for c in range(NHT)]
    xtok = [xtok_a[:, ts(j, D)] for j in range(NT)]

    # ---- constants ----
    eps_t = mt([P, 1], "eps", F32)
    nc.gpsimd.memset(eps_t[:], EPS)
    onesf = mt([P, P], "onesf", F32)
    nc.gpsimd.memset(onesf[:], 1.0)
    ones_bf = mt([P, P], "onesbf")
    nc.vector.tensor_copy(ones_bf[:], onesf[:])
    band = mt([P, 896], "band", F32)
    nc.gpsimd.memset(band[:], 1.0)
    nc.gpsimd.affine_select(out=band[:], in_=band[:],
                            compare_op=mybir.AluOpType.is_ge,
                            fill=0.0, base=-384, channel_multiplier=-1,
                            pattern=[[1, 896]])
    band_bf = mt([P, 896], "bandbf")
    nc.vector.tensor_copy(band_bf[:], band[:])

    def scale_row(ms_row, s_row, s_bf_row):
        # rms = sqrt(ms/D + eps); s = 1/rms
        nc.scalar.activation(s_row[0:1, :], ms_row[0:1, :], AF.Sqrt,
                             scale=1.0 / D, bias=eps_t[0:1, 0:1])
        nc.vector.reciprocal_approx_fast(s_row[0:1, :], s_row[0:1, :])
        nc.vector.tensor_copy(s_bf_row[0:1, :], s_row[0:1, :])

    def bcast_chunk(s_bf_row, s_bcast, ch):
        bb = paux.tile([P, CH], F32, tag="aux", name="bb")
        nc.tensor.matmul(bb[:], ones_bf[0:1, :], s_bf_row[0:1, ts(ch, CH)],
                         start=True, stop=True)
        nc.vector.tensor_copy(s_bcast[:, ts(ch, CH)], bb[:])

    # ---- norm1 -> HT -> K^T/Q^T, pipelined per 512-token chunk ----
    s1_row = mt([1, T], "s1r", F32)
    s1_bf = mt([1, T], "s1bf")
    s1b = mt([P, T], "s1b")
    ht = [mt([P, T], f"ht{c}") for c in range(ND)]
    kt = [mt([P, T], f"kt{c}") for c in range(ND)]
    qt = [mt([P, T], f"qt{c}") for c in range(ND)]
    for ch in range(NCH):
        sl = ts(ch, CH)
        sqs = []
        for c in range(ND):
            t = pscr.tile([P, CH], BF16, tag="nsq", name=f"nsq{c}")
            nc.vector.tensor_mul(t[:], xtr[c][:, sl], xtr[c][:, sl])
            sqs.append(t)
        ms = paux.tile([P, CH], F32, tag="aux", name="ms")
        for c in range(ND):
            nc.tensor.matmul(ms[0:1, :], ones_bf[:, 0:1], sqs[c][:],
                             start=(c == 0), stop=(c == ND - 1))
        nc.scalar.activation(s1_row[0:1, sl], ms[0:1, :], AF.Sqrt,
                             scale=1.0 / D, bias=eps_t[0:1, 0:1])
        nc.vector.reciprocal_approx_fast(s1_row[0:1, sl], s1_row[0:1, sl])
        nc.vector.tensor_copy(s1_bf[0:1, sl], s1_row[0:1, sl])
        bcast_chunk(s1_bf, s1b, ch)
        for c in range(ND):
            nc.vector.tensor_mul(ht[c][:, sl], xtr[c][:, sl], s1b[:, sl])
        for dst, w_s in ((kt, wk_s), (qt, wq_s)):
            for dt in range(ND):
                ps = paux.tile([P, CH], F32, tag="aux", name="proj")
                for c in range(ND):
                    nc.tensor.matmul(ps[:], w_s[c][:, ts(dt, P)],
                                     ht[c][:, sl],
                                     start=(c == 0), stop=(c == ND - 1))
                nc.scalar.copy(dst[dt][:, sl], ps[:])
    # preload the exp table set before the first score exp
    dummy = mt([1, 1], "dummy", F32)
    nc.scalar.activation(dummy[0:1, :], eps_t[0:1, 0:1], AF.Exp)

    vaug = [mt([P, NH * SLOT], f"va{j}") for j in range(NT)]

    def v_proj(j):
        nc.vector.tensor_copy(
            vaug[j][:].rearrange("p (h e) -> p h e", h=NH)[:, :, HD : SLOT],
            onesf[:, 0:NH].rearrange("p (h e) -> p h e", e=1),
        )
        ps = paux.tile([P, CH], F32, tag="aux", name="vproj")
        for c in range(ND):
            nc.tensor.matmul(ps[:, 0:D], ht[c][:, ts(j, P)], wv_s[c][:],
                             start=(c == 0), stop=(c == ND - 1))
        nc.vector.tensor_copy(
            vaug[j][:].rearrange("p (h e) -> p h e", h=NH)[:, :, 0 : HD],
            ps[:, 0:D].rearrange("p (h e) -> p h e", h=NH),
        )

    # ---- attention + x2^T + norm2 stats ----
    ot = [mt([P, T], f"ot{c}") for c in range(ND)]
    x2t = [mt([P, T], f"x2t{c}") for c in range(ND)]
    ms2_row = mt([1, T], "ms2", F32)

    psS_cm = tc.tile_pool(name="psS", bufs=1, space="PSUM")
    psS = psS_cm.__enter__()
    psO_cm = tc.tile_pool(name="psO", bufs=1, space="PSUM")
    psO = psO_cm.__enter__()

    def exp_tile(p_sb, s_ps, ch, kt0):
        """exp over an [P, 2CH] score pair; causal masking via in-place
        band multiply (dead cells hold exp(garbage), finite, zeroed by
        the 0/1 band)."""
        nc.scalar.activation(p_sb[:], s_ps[:], AF.Exp, scale=SCL)
        for m in range(2):
            d = (kt0 + m) * P - ch * CH
            if d >= 0:
                nc.vector.tensor_mul(p_sb[:, ts(m, CH)], p_sb[:, ts(m, CH)],
                                     band_bf[:, 384 - d : 896 - d])

    def qk_pair(st, ch, par):
        """Emit the K=64 row-tiled QK matmuls for one head of stage st."""
        dt, kt0 = st
        lo, hi = (0, HD) if par == 0 else (HD, P)
        s_ps = psS.tile([P, 2 * CH], F32, tag=("se" if par == 0 else "so"),
                        name="s")
        for m in range(2):
            nc.tensor.matmul(s_ps[:, ts(m, CH)],
                             kt[dt][lo:hi, ts(kt0 + m, P)],
                             qt[dt][lo:hi, ts(ch, CH)], start=True, stop=True)
        return s_ps

    def attn_chunk(ch):
        ntk = 4 * (ch + 1)
        sl = ts(ch, CH)
        stages = [(dt, kt0) for dt in range(ND) for kt0 in range(0, ntk, 2)]
        o_ps = {}
        s_cur = {}
        s_cur[0] = qk_pair(stages[0], ch, 0)
        s_cur[1] = qk_pair(stages[0], ch, 1)
        for i, (dt, kt0) in enumerate(stages):
            if kt0 == 0:
                o_ps[(dt, 0)] = psO.tile([P, CH], F32, tag="oe", name="oe")
                o_ps[(dt, 1)] = psO.tile([P, CH], F32, tag="oo", name="oo")
            nxt = stages[i + 1] if i + 1 < len(stages) else None
            p_sb = {}
            s_stage = (s_cur[0], s_cur[1])
            for par in range(2):
                p_sb[par] = patt.tile([P, 2 * CH], BF16,
                                      tag=("pe" if par == 0 else "po"),
                                      name="p")
                exp_tile(p_sb[par], s_stage[par], ch, kt0)
                if nxt is not None:
                    s_cur[par] = qk_pair(nxt, ch, par)
            for par in range(2):
                h = 2 * dt + par
                for m in range(2):
                    k = kt0 + m
                    nc.tensor.matmul(
                        o_ps[(dt, par)][0:SLOT, :],
                        vaug[k][:, h * SLOT : (h + 1) * SLOT],
                        p_sb[par][:, ts(m, CH)],
                        start=(k == 0), stop=(k == ntk - 1))
            if kt0 + 2 >= ntk:
                # last k-pair of this head pair: normalize both heads
                for par in range(2):
                    lo, hi = (0, HD) if par == 0 else (HD, P)
                    rz = pnrm.tile([P, CH], F32, tag="rz", name="rz")
                    nc.vector.tensor_copy(rz[0:1, :],
                                          o_ps[(dt, par)][HD : HD + 1, :])
                    nc.vector.reciprocal_approx_fast(rz[0:1, :], rz[0:1, :])
                    rzb = pnrm.tile([P, CH], F32, tag="rzb", name="rzb")
                    nc.gpsimd.partition_broadcast(rzb[0:HD, :], rz[0:1, :])
                    nc.vector.tensor_mul(ot[dt][lo:hi, sl],
                                         o_ps[(dt, par)][0:HD, :],
                                         rzb[0:HD, :])
        # x2^T chunk = x^T + wo^T @ o^T ; norm2 stats for this chunk
        sqs = []
        for dt2 in range(ND):
            xx = paux.tile([P, CH], F32, tag="aux", name="xx")
            for c in range(ND):
                nc.tensor.matmul(xx[:], wo_s[c][:, ts(dt2, P)], ot[c][:, sl],
                                 start=(c == 0), stop=(c == ND - 1))
            nc.vector.tensor_add(x2t[dt2][:, sl], xx[:], xtr[dt2][:, sl])
            sq = pscr.tile([P, CH], BF16, tag="sq2", name="sq2")
            nc.vector.tensor_mul(sq[:], x2t[dt2][:, sl], x2t[dt2][:, sl])
            sqs.append(sq)
        ms2 = paux.tile([P, CH], F32, tag="aux", name="ms2")
        for dt2 in range(ND):
            nc.tensor.matmul(ms2[0:1, :], ones_bf[:, 0:1], sqs[dt2][:],
                             start=(dt2 == 0), stop=(dt2 == ND - 1))
        nc.vector.tensor_copy(ms2_row[0:1, sl], ms2[0:1, :])

    for j in range(4):
        v_proj(j)
    for ch in range(NCH):
        attn_chunk(ch)
        if ch < NCH - 1:
            for j in range(4 * (ch + 1), 4 * (ch + 2)):
                v_proj(j)

    psO_cm.__exit__(None, None, None)
    psS_cm.__exit__(None, None, None)

    # ---- norm2 scale (batched: single table switch) ----
    s2_row = mt([1, T], "s2r", F32)
    s2_bf = mt([1, T], "s2bf")
    s2b = mt([P, T], "s2b")
    scale_row(ms2_row, s2_row, s2_bf)

    psF_cm = tc.tile_pool(name="psF", bufs=2, space="PSUM")
    psF = psF_cm.__enter__()
    psG_cm = tc.tile_pool(name="psG", bufs=2, space="PSUM")
    psG = psG_cm.__enter__()

    # h2t reuses the (dead) ht slots
    h2t = [main.tile([P, T], BF16, tag=f"ht{c}", name=f"h2t{c}")
           for c in range(ND)]
    for ch in range(NCH):
        bcast_chunk(s2_bf, s2b, ch)
        for c in range(ND):
            nc.vector.tensor_mul(h2t[c][:, ts(ch, CH)], x2t[c][:, ts(ch, CH)],
                                 s2b[:, ts(ch, CH)])

    # ---- FFN + output (gt reuses qt/kt slots) ----
    gt = [main.tile([P, T], BF16, tag=(f"qt{c}" if c < ND else f"kt{c - ND}"),
                    name=f"gt{c}") for c in range(NHT)]
    for half in range(NCH // 2):
        for htile in range(NHT):
            ps = psF.tile([P, 2 * CH], F32, tag="a1", name="a1")
            for m in range(2):
                for c in range(ND):
                    nc.tensor.matmul(ps[:, ts(m, CH)],
                                     fw1_s[c][:, ts(htile, P)],
                                     h2t[c][:, ts(2 * half + m, CH)],
                                     start=(c == 0), stop=(c == ND - 1))
            nc.scalar.activation(gt[htile][:, ts(half, 2 * CH)], ps[:],
                                 AF.Gelu, bias=b1_s[:, htile : htile + 1])

        for j in range(8 * half, 8 * (half + 1)):
            ps = psG.tile([P, D], F32, tag="g", name="g")
            for c in range(ND):
                nc.tensor.matmul(ps[:], ot[c][:, ts(j, P)], wo_s[c][:],
                                 start=(c == 0), stop=False)
            for c in range(NHT):
                nc.tensor.matmul(ps[:], gt[c][:, ts(j, P)], fw2_s[c][:],
                                 start=False, stop=False)
            nc.tensor.matmul(ps[:], ones_bf[0:1, :], b2_row[0:1, :],
                             start=False, stop=True)
            o_t = pout.tile([P, D], F32, tag="o", name="o")
            nc.vector.tensor_add(o_t[:], ps[:], xtok[j])
            nc.sync.dma_start(out_d[ts(j, P), :], o_t[:])

    psG_cm.__exit__(None, None, None)
    psF_cm.__exit__(None, None, None)
    paux_cm.__exit__(None, None, None)
    pout_cm.__exit__(None, None, None)
    pnrm_cm.__exit__(None, None, None)
    patt_cm.__exit__(None, None, None)
    pscr_cm.__exit__(None, None, None)
    main_cm.__exit__(None, None, None)


_CACHE = {}


def _build():
    if "nc" in _CACHE:
        return _CACHE["nc"]
    nc = bacc.Bacc("TRN2", target_bir_lowering=False, debug=False)
    din = {}
    for name, shape, dt_ in (
        ("xtok", [T, D], BF16), ("xtr", [D, T], BF16),
        ("wq", [D, D], BF16), ("wk", [D, D], BF16),
        ("wv", [D, D], BF16), ("wo", [D, D], BF16),
        ("fw1", [D, HDIM], BF16), ("fb1", [HDIM], F32),
        ("fw2", [HDIM, D], BF16), ("fb2", [D], BF16),
    ):
        din[name] = nc.dram_tensor(name, shape, dt_, kind="ExternalInput").ap()
    out_d = nc.dram_tensor("out", [T, D], F32, kind="ExternalOutput").ap()
    with tile.TileContext(nc) as tc:
        _body(tc, din, out_d)
    nc.compile()
    _CACHE["nc"] = nc
    return nc


def run(inputs: dict, trace: bool = False):
    """Run on 8 cores; returns (output [8,T,D], BassKernelResults)."""
    nc = _build()
    x = np.asarray(inputs["x"], dtype=np.float32)
    ln1 = np.asarray(inputs["ln1_w"], dtype=np.float32)
    ln2 = np.asarray(inputs["ln2_w"], dtype=np.float32)
    bf = ml_dtypes.bfloat16
    shared = {
        "wq": (ln1[:, None] * np.asarray(inputs["wq"], np.float32)).astype(bf),
        "wk": (ln1[:, None] * np.asarray(inputs["wk"], np.float32)).astype(bf),
        "wv": (ln1[:, None] * np.asarray(inputs["wv"], np.float32)).astype(bf),
        "wo": np.asarray(inputs["wo"], np.float32).astype(bf),
        "fw1": (ln2[:, None] * np.asarray(inputs["ff_w1"], np.float32)).astype(bf),
        "fb1": np.asarray(inputs["ff_b1"], np.float32),
        "fw2": np.asarray(inputs["ff_w2"], np.float32).astype(bf),
        "fb2": np.asarray(inputs["ff_b2"], np.float32).astype(bf),
    }
    shared = {k: np.ascontiguousarray(v) for k, v in shared.items()}
    in_maps = [
        dict(shared,
             xtok=np.ascontiguousarray(x[c].astype(bf)),
             xtr=np.ascontiguousarray(x[c].T.astype(bf)))
        for c in range(NCORES)
    ]
    res = run_bass_kernel_spmd(nc, in_maps, list(range(NCORES)), trace=trace)
    out = np.stack([res.results[c]["out"] for c in range(NCORES)], axis=0)
    return out, res


def kernel(**inputs) -> np.ndarray:
    out, _ = run(inputs, trace=False)
    return out
